# revision 1
# baseline (speedup 1.0000x reference)
"""CRF negative-log-likelihood kernel for Trainium2, SPMD over 8 NeuronCores.

Strategy
--------
Data-parallel over batch: core c handles sequences b in [c*8, (c+1)*8).

Per core (B=8 local sequences, T=512, K=50 tags, D=1024), all fp32:

1. Emissions GEMM in transposed layout emisT[k, bt]:  TensorE contracts
   the partition dim, so the moving operand must be hidden^T.  hidden is
   HWDGE-loaded, transposed 128x128-wise on the TensorE (identity
   matmul, PSUM out), copied PSUM->SBUF by DVE, then the GEMM
   accumulates 8 d-chunks with W (already d-major in DRAM) stationary.
2. Partition function: linear-domain forward recurrence
       alpha_t = (expT^T @ alpha_{t-1}) * E_t
   with E = exp(emisT + b).  Two independent chains (sequences 0-3 on
   partitions 0-49, 4-7 on partitions 64-113 via PE row/col groups)
   overlap each other's PE<->DVE latency.  Every RENORM steps a rank-1
   matmul sums alpha; the reciprocal is broadcast back over partitions
   with another rank-1 matmul and folded into the E column two steps
   ahead (scale propagates linearly); log(sum) accumulates into C.
   log_Z = log(sum_j alpha_T) + C, with exp(end_trans) pre-folded into
   the last E column and exp(start_trans) into alpha_0.
3. Gold path score via one-hot algebra (no gathers):
       OH[k, (b,t)] = (k == tag[b,t])       (iota compare of a rank-1
                                             broadcast matmul of tags)
       R[k, (b,t)]  = trans[tag[b,t-1], k]  (matmul: trans^T @ OH shifted)
       G = emisT + b + R, G[:,b,0] += start, G[:,b,511] += end
       gold[b] = sum_{k,t} G * OH           (DVE mul+reduce, ones matmul)
4. out[b] = log_Z[b] - gold[b].
"""

import numpy as np

B_FULL = 64
B_LOC = 8
BH = 4  # sequences per chain
T = 512
K = 50
D = 1024
BT = B_LOC * T  # 4096
N_CORES = 8
D_CHUNKS = D // 128  # 8
RENORM = 8
H2 = 64  # partition base of chain B

_COMPILED = {}
LAST_RESULT = None


def _build(dbg=False):
    import concourse.bass as bass
    import concourse.tile as tile
    from concourse import bacc, mybir

    f32 = mybir.dt.float32

    nc = bacc.Bacc(
        "TRN2",
        target_bir_lowering=False,
        debug=False,
        num_devices=N_CORES,
    )

    hid = nc.dram_tensor("hid", [BT, D], f32, kind="ExternalInput")
    wq = nc.dram_tensor("wq", [D_CHUNKS, 128, K], f32, kind="ExternalInput")
    ident = nc.dram_tensor("ident", [128, 128], f32, kind="ExternalInput")
    # doubled constants: rows [0:50] chain A, rows [64:114] chain B
    expT2 = nc.dram_tensor("expT2", [128, K], f32, kind="ExternalInput")
    transr2 = nc.dram_tensor("transr2", [128, K], f32, kind="ExternalInput")
    cols2 = nc.dram_tensor("cols2", [128, 7], f32, kind="ExternalInput")
    # cols2 columns: 0=expstart 1=expend 2=startc 3=endc 4=bcol 5=iota 6=ones
    tagrow = nc.dram_tensor("tagrow", [1, BT], f32, kind="ExternalInput")
    onesr = nc.dram_tensor("onesr", [1, K], f32, kind="ExternalInput")
    out_d = nc.dram_tensor("out", [1, B_LOC], f32, kind="ExternalOutput")
    if dbg:
        dbg_e = nc.dram_tensor("dbg_e", [K, 13], f32, kind="ExternalOutput")
        dbg_ht = nc.dram_tensor("dbg_ht", [128, 16], f32, kind="ExternalOutput")
        dbg_gold = nc.dram_tensor("dbg_gold", [1, B_LOC], f32, kind="ExternalOutput")
        dbg_c = nc.dram_tensor("dbg_c", [1, B_LOC], f32, kind="ExternalOutput")
        dbg_lnz = nc.dram_tensor("dbg_lnz", [1, B_LOC], f32, kind="ExternalOutput")
        dbg_al1a = nc.dram_tensor("dbg_al1a", [K, BH], f32, kind="ExternalOutput")
        dbg_al1b = nc.dram_tensor("dbg_al1b", [K, BH], f32, kind="ExternalOutput")

    AF = mybir.ActivationFunctionType
    ALU = mybir.AluOpType
    AX = mybir.AxisListType

    with tile.TileContext(nc) as tc:
        with (
            tc.tile_pool(name="consts", bufs=1) as consts,
            tc.tile_pool(name="hnat", bufs=2) as hnat_pool,
            tc.tile_pool(name="ht", bufs=2) as ht_pool,
            tc.tile_pool(name="persist", bufs=1) as persist,
            tc.tile_pool(name="small", bufs=4) as small,
            tc.tile_pool(name="alpha", bufs=3) as alpha_pool,
            tc.tile_pool(name="tp_psum", bufs=2, space=bass.MemorySpace.PSUM) as tpsum,
            tc.tile_pool(name="big_psum", bufs=2, space=bass.MemorySpace.PSUM) as bpsum,
            tc.tile_pool(name="scan_psum", bufs=3, space=bass.MemorySpace.PSUM) as spsum,
        ):
            # ---- constants ----
            w_sb = consts.tile([128, D_CHUNKS, K], f32)
            nc.scalar.dma_start(w_sb[:], wq[:].rearrange("c p k -> p c k"))
            id_sb = consts.tile([128, 128], f32)
            nc.scalar.dma_start(id_sb[:], ident[:])
            expT_sb = consts.tile([128, K], f32)
            nc.scalar.dma_start(expT_sb[:], expT2[:])
            transr_sb = consts.tile([128, K], f32)
            nc.scalar.dma_start(transr_sb[:], transr2[:])
            cols_sb = consts.tile([128, 7], f32)
            nc.scalar.dma_start(cols_sb[:], cols2[:])
            tag_sb = consts.tile([1, BT], f32)
            nc.scalar.dma_start(tag_sb[:], tagrow[:])
            onesr_sb = consts.tile([1, K], f32)
            nc.scalar.dma_start(onesr_sb[:], onesr[:])

            expstart = cols_sb[:, 0:1]
            expend = cols_sb[:, 1:2]
            startc = cols_sb[:, 2:3]
            endc = cols_sb[:, 3:4]
            bcol = cols_sb[:, 4:5]
            iota = cols_sb[:, 5:6]
            onesc = cols_sb[:, 6:7]

            # persistent per-chain tensors; chain B lives at partitions 64:114
            E_a = persist.tile([K, BH, T], f32)
            E_bf = persist.tile([128, BH, T], f32)
            emis_a = persist.tile([K, BH, T], f32)
            emis_bf = persist.tile([128, BH, T], f32)
            oh_a = persist.tile([K, BH, T], f32)
            oh_bf = persist.tile([128, BH, T], f32)

            def half(c):
                """(row slice lo, chain tensors) for local sequence c."""
                if c < BH:
                    return 0, E_a, emis_a, oh_a, c
                return H2, E_bf, emis_bf, oh_bf, c - BH

            # ---- phase B: load + PE transpose + emissions GEMM ----
            for c in range(B_LOC):
                lo, E_t, em_t, _, a = half(c)
                hnat = hnat_pool.tile([128, 4, D], f32, tag="hnat")
                src = hid[c * T : (c + 1) * T, :].rearrange("(a p) d -> p a d", p=128)
                nc.sync.dma_start(hnat[:], src)

                ht = ht_pool.tile([128, D_CHUNKS, T], f32, tag="ht")
                for aa in range(4):
                    for dc in range(D_CHUNKS):
                        pst = tpsum.tile([128, 128], f32, tag="tp")
                        nc.tensor.transpose(
                            pst[:], hnat[:, aa, dc * 128 : (dc + 1) * 128], id_sb[:]
                        )
                        nc.vector.tensor_copy(
                            ht[:, dc, aa * 128 : (aa + 1) * 128], pst[:]
                        )

                if dbg and c == 0:
                    nc.sync.dma_start(dbg_ht[:], ht[:, 0, 0:16])
                ps = bpsum.tile([128, T], f32, tag="big")
                for dc in range(D_CHUNKS):
                    nc.tensor.matmul(
                        ps[lo : lo + K, :],
                        w_sb[:, dc, :],
                        ht[:, dc, :],
                        start=(dc == 0),
                        stop=(dc == D_CHUNKS - 1),
                    )
                nc.scalar.activation(
                    E_t[lo : lo + K, a, :], ps[lo : lo + K, :], AF.Exp,
                    bias=bcol[lo : lo + K],
                )
                nc.scalar.activation(
                    em_t[lo : lo + K, a, :], ps[lo : lo + K, :], AF.Identity,
                    bias=bcol[lo : lo + K],
                )

            if dbg:
                nc.sync.dma_start(dbg_e[:], E_a[0:K, 0, 0:13])
            # ---- phase C: gold score ----
            for c in range(B_LOC):
                lo, _, _, oh_t, a = half(c)
                psb = bpsum.tile([128, T], f32, tag="big")
                nc.tensor.matmul(
                    psb[lo : lo + K, :], onesr_sb[:],
                    tag_sb[:, c * T : (c + 1) * T], start=True, stop=True,
                )
                nc.vector.tensor_scalar(
                    oh_t[lo : lo + K, a, :], psb[lo : lo + K, :],
                    iota[lo : lo + K], None, ALU.is_equal,
                )
            for c in range(B_LOC):
                lo, _, em_t, oh_t, a = half(c)
                psc = bpsum.tile([128, T], f32, tag="big")
                nc.tensor.matmul(
                    psc[lo : lo + K, 0 : T - 1],
                    transr_sb[lo : lo + K, :],
                    oh_t[lo : lo + K, a, 0 : T - 1],
                    start=True, stop=True,
                )
                nc.vector.tensor_add(
                    em_t[lo : lo + K, a, 1:T],
                    em_t[lo : lo + K, a, 1:T],
                    psc[lo : lo + K, 0 : T - 1],
                )
            for lo, em_t, oh_t in ((0, emis_a, oh_a), (H2, emis_bf, oh_bf)):
                sl = slice(lo, lo + K)
                nc.vector.tensor_scalar_add(
                    em_t[sl, :, 0], em_t[sl, :, 0], startc[sl]
                )
                nc.vector.tensor_scalar_add(
                    em_t[sl, :, T - 1], em_t[sl, :, T - 1], endc[sl]
                )
                nc.vector.tensor_mul(oh_t[sl, :, :], oh_t[sl, :, :], em_t[sl, :, :])
            goldkb_a = persist.tile([K, BH], f32)
            goldkb_bf = persist.tile([128, BH], f32)
            nc.vector.tensor_reduce(goldkb_a[:], oh_a[0:K], AX.X, ALU.add)
            nc.vector.tensor_reduce(
                goldkb_bf[H2 : H2 + K], oh_bf[H2 : H2 + K], AX.X, ALU.add
            )
            gold_sb = small.tile([1, B_LOC], f32, tag="row")
            gps_a = bpsum.tile([1, BH], f32, tag="big")
            nc.tensor.matmul(
                gps_a[:], onesc[0:K], goldkb_a[:], start=True, stop=True
            )
            nc.scalar.copy(gold_sb[:, 0:BH], gps_a[:])
            gps_b = bpsum.tile([1, BH], f32, tag="big")
            nc.tensor.matmul(
                gps_b[:], onesc[H2 : H2 + K], goldkb_bf[H2 : H2 + K],
                start=True, stop=True,
            )
            nc.scalar.copy(gold_sb[:, BH:B_LOC], gps_b[:])
            if dbg:
                nc.sync.dma_start(dbg_gold[:], gold_sb[:])

            # ---- phase D: forward scan, two chains ----
            c_sb = persist.tile([1, B_LOC], f32)
            nc.vector.memset(c_sb[:], 0.0)
            for lo, E_t in ((0, E_a), (H2, E_bf)):
                sl = slice(lo, lo + K)
                nc.vector.tensor_scalar_mul(
                    E_t[sl, :, T - 1], E_t[sl, :, T - 1], expend[sl]
                )
            alpha_a = alpha_pool.tile([K, BH], f32, tag="aa")
            nc.vector.tensor_scalar_mul(alpha_a[:], E_a[0:K, :, 0], expstart[0:K])
            alpha_bf = alpha_pool.tile([128, BH], f32, tag="ab")
            slb = slice(H2, H2 + K)
            nc.vector.tensor_scalar_mul(
                alpha_bf[slb], E_bf[slb, :, 0], expstart[slb]
            )

            chains = [
                # (row-lo, E tile, alpha AP getter, alpha tag, C cols)
                [0, E_a, alpha_a[:], "aa", slice(0, BH)],
                [H2, E_bf, alpha_bf[slb], "ab", slice(BH, B_LOC)],
            ]

            for t in range(1, T):
                do_renorm = (t % RENORM == 0) and (t + 2 < T - 1)
                for ch in chains:
                    lo, E_t, alpha_ap, atag, ccols = ch
                    sl = slice(lo, lo + K)
                    ps = spsum.tile([128, BH], f32, tag="scan", name=f"ps{t}_{lo}")
                    nc.tensor.matmul(
                        ps[sl], expT_sb[sl], alpha_ap, start=True, stop=True
                    )
                    if do_renorm:
                        # side chain: s = sum(alpha_{t-1}); E[t+2] *= 1/s; C += ln s
                        sps = spsum.tile([1, BH], f32, tag="ssum", bufs=1, name=f"ss{t}_{lo}")
                        nc.tensor.matmul(
                            sps[:], onesc[sl], alpha_ap, start=True, stop=True
                        )
                        r_sb = small.tile([1, BH], f32, tag="row")
                        nc.vector.reciprocal(r_sb[:], sps[:])
                        psr = spsum.tile([128, BH], f32, tag="scan", name=f"pr{t}_{lo}")
                        nc.tensor.matmul(
                            psr[sl], onesr_sb[:], r_sb[:], start=True, stop=True
                        )
                        nc.vector.tensor_mul(
                            E_t[sl, :, t + 2], E_t[sl, :, t + 2], psr[sl]
                        )
                        lns = small.tile([1, BH], f32, tag="row")
                        nc.scalar.activation(lns[:], sps[:], AF.Ln)
                        nc.vector.tensor_add(
                            c_sb[:, ccols], c_sb[:, ccols], lns[:]
                        )
                    if lo == 0:
                        alpha_new = alpha_pool.tile([K, BH], f32, tag=atag)
                        new_ap = alpha_new[:]
                    else:
                        alpha_new = alpha_pool.tile([128, BH], f32, tag=atag)
                        new_ap = alpha_new[slb]
                    nc.vector.tensor_mul(new_ap, ps[sl], E_t[sl, :, t])
                    if dbg and t == 1:
                        nc.sync.dma_start(
                            dbg_al1a[:] if lo == 0 else dbg_al1b[:], new_ap
                        )
                    ch[2] = new_ap

            lnz = small.tile([1, B_LOC], f32, tag="row")
            for ch in chains:
                lo, E_t, alpha_ap, atag, ccols = ch
                sl = slice(lo, lo + K)
                zps = spsum.tile([1, BH], f32, tag="ssum", bufs=1, name=f"z{lo}")
                nc.tensor.matmul(zps[:], onesc[sl], alpha_ap, start=True, stop=True)
                nc.scalar.activation(lnz[:, ccols], zps[:], AF.Ln)
            if dbg:
                nc.sync.dma_start(dbg_c[:], c_sb[:])
                nc.sync.dma_start(dbg_lnz[:], lnz[:])
            nc.vector.tensor_add(lnz[:], lnz[:], c_sb[:])
            outrow = small.tile([1, B_LOC], f32, tag="row")
            nc.vector.tensor_sub(outrow[:], lnz[:], gold_sb[:])
            nc.sync.dma_start(out_d[:], outrow[:])

    nc.compile()
    return nc


def _get_compiled():
    if "nc" not in _COMPILED:
        _COMPILED["nc"] = _build()
    return _COMPILED["nc"]


def _doubled(col):
    """[50] -> [128] with copies at rows 0:50 and 64:114."""
    v = np.zeros(128, np.float32)
    v[0:K] = col
    v[H2 : H2 + K] = col
    return v


def kernel(full_hidden, tag_ids, mask, W, b, transitions, start_trans, end_trans):
    global LAST_RESULT
    from concourse.bass_utils import run_bass_kernel_spmd

    full_hidden = np.ascontiguousarray(np.asarray(full_hidden, dtype=np.float32))
    tags = np.asarray(tag_ids)
    W = np.asarray(W, dtype=np.float32)
    b = np.asarray(b, dtype=np.float32)
    transitions = np.asarray(transitions, dtype=np.float32)
    start_trans = np.asarray(start_trans, dtype=np.float32)
    end_trans = np.asarray(end_trans, dtype=np.float32)

    nc = _get_compiled()

    expT2 = np.zeros((128, K), np.float32)
    expT2[0:K] = np.exp(transitions)
    expT2[H2 : H2 + K] = np.exp(transitions)
    transr2 = np.zeros((128, K), np.float32)
    transr2[0:K] = transitions
    transr2[H2 : H2 + K] = transitions
    cols2 = np.stack(
        [
            _doubled(np.exp(start_trans)),
            _doubled(np.exp(end_trans)),
            _doubled(start_trans),
            _doubled(end_trans),
            _doubled(b),
            _doubled(np.arange(K, dtype=np.float32)),
            _doubled(np.ones(K, np.float32)),
        ],
        axis=1,
    ).astype(np.float32)

    common = {
        "wq": np.ascontiguousarray(W.reshape(D_CHUNKS, 128, K)),
        "ident": np.eye(128, dtype=np.float32),
        "expT2": expT2,
        "transr2": transr2,
        "cols2": np.ascontiguousarray(cols2),
        "onesr": np.ones((1, K), np.float32),
    }
    in_maps = []
    for c in range(N_CORES):
        sl = slice(c * B_LOC, (c + 1) * B_LOC)
        in_maps.append(
            {
                "hid": np.ascontiguousarray(full_hidden[sl].reshape(BT, D)),
                "tagrow": np.ascontiguousarray(
                    tags[sl].astype(np.float32).reshape(1, BT)
                ),
                **common,
            }
        )

    res = run_bass_kernel_spmd(nc, in_maps, core_ids=list(range(N_CORES)))
    LAST_RESULT = res
    out = np.concatenate(
        [np.asarray(res.results[c]["out"]).reshape(B_LOC) for c in range(N_CORES)]
    )
    return out.astype(np.float32)

